# revision 23
# baseline (speedup 1.0000x reference)
"""Sparse attention kernel for Trainium2 (8 NeuronCores, data-parallel over batch).

Reference computation (per batch row b):
    q    = x @ q_w.T                                  [N, C]
    xkv  = x[key_ind]                                 [NKV, C]
    kv   = xkv @ kv_w.T -> per-head k, v              [NKV, 2C]
    attn = softmax((q*scale) @ k.T) @ v               [N, C]
    out  = attn @ proj_w.T + proj_b                   [N, C]

Compute plan (per core = one batch row):
  - All dense projections (q/k/v/proj) run as 3-term hi/lo fp8 DoubleRow
    chains: (a_hi + a_lo) @ (w_hi + w_lo) with the lo*lo term dropped.
    Each DoubleRow matmul contracts TWO 128-deep k-chunks at 0.5 cyc/row,
    so a 768-deep contraction runs in 9 DR matmuls = 0.75x the bf16 cost
    at near-bf16 accuracy. Weights are pre-scaled x32 host-side so their
    hi/lo planes stay clear of e4m3's subnormal range; descales fold into
    the exp scale and the evac ops.
  - Scores: one DoubleRow matmul per key-tile with stationary (k_hi,k_lo)
    and the moving q8 duplicated via a stride-0 AP (2x PE throughput; the
    single-fp8 q quantization is the only real precision loss, ~1.4e-2).
  - OV runs transposed: stationary = p-tile [keys, 128tok], moving =
    vaug [keys, 65] (v columns + a 1/16 ones column) -> out [tok, 65]
    costs 65 cycles/matmul instead of 512. Denominator lands in col 64;
    normalization is a batched strided reciprocal + stride-0-broadcast
    multiply on DVE producing at16 = 16*attn (x16 keeps the hi/lo split
    of attn clear of subnormals).
  - at16 [tok, feat] returns to [feat, tok] via XBAR transpose DMA
    (14ns per 16x128 tile, on the DMA queues, zero engine cost), then
    Pool splits it into fp8 hi/lo planes for the proj chains.
  - Engine balance: Act does ONLY the exp stream (its ~82us floor
    dominates), all PSUM evacs are DVE, all SBUF-side casts are Pool
    (Pool cannot touch PSUM). Deferred OV/cast/proj work from chunk ch
    drains through the 12 (head) slots of chunk ch+1's score loop.
"""
import os
import sys

sys.path.insert(0, "/opt/trn_rl_repo")

import numpy as np  # noqa: E402
import ml_dtypes  # noqa: E402

B, N, C = 8, 2048, 768
NKV = 512
H = 12
HD = C // H          # 64
SCALE = HD ** -0.5
P = 128
CT = C // P          # 6 feature tiles
NC2 = 512            # token chunk
NCH = N // NC2       # 4 chunks
MCH = NKV // P       # 4 key chunks
G = H // 2           # 6 head pairs
WS = 32.0            # weight pre-scale (host side)
AS = 16.0            # attn pre-scale (via 1/16 ones column)

_CACHE = {}


def _build():
    import concourse.bass as bass
    import concourse.mybir as mybir
    import concourse.tile as tile
    from concourse import bacc
    from contextlib import ExitStack

    F32 = mybir.dt.float32
    BF16 = mybir.dt.bfloat16
    E4 = mybir.dt.float8e4
    I16 = mybir.dt.int16
    Exp = mybir.ActivationFunctionType.Exp
    DR = mybir.MatmulPerfMode.DoubleRow
    Mult = mybir.AluOpType.mult
    Add = mybir.AluOpType.add
    Sub = mybir.AluOpType.subtract

    nc = bacc.Bacc("TRN2", target_bir_lowering=False, debug=False, num_devices=8)

    xTh = nc.dram_tensor("xTh", [NCH, P, CT * NC2], E4, kind="ExternalInput")
    xTl = nc.dram_tensor("xTl", [NCH, P, CT * NC2], E4, kind="ExternalInput")
    xr = nc.dram_tensor("xr", [N, C], BF16, kind="ExternalInput")
    idx = nc.dram_tensor("idx", [P, NKV // 16], I16, kind="ExternalInput")
    wqh = nc.dram_tensor("wqh", [G, P, CT * P], E4, kind="ExternalInput")
    wql = nc.dram_tensor("wql", [G, P, CT * P], E4, kind="ExternalInput")
    wkh = nc.dram_tensor("wkh", [P, CT * C], E4, kind="ExternalInput")
    wkl = nc.dram_tensor("wkl", [P, CT * C], E4, kind="ExternalInput")
    wvh = nc.dram_tensor("wvh", [P, CT * C], E4, kind="ExternalInput")
    wvl = nc.dram_tensor("wvl", [P, CT * C], E4, kind="ExternalInput")
    wph = nc.dram_tensor("wph", [P, CT * C], E4, kind="ExternalInput")
    wpl = nc.dram_tensor("wpl", [P, CT * C], E4, kind="ExternalInput")
    pb = nc.dram_tensor("pb", [P, CT], F32, kind="ExternalInput")
    out = nc.dram_tensor("out", [P, CT, NCH, NC2], BF16, kind="ExternalOutput")

    def dup2(ap):
        # moving-pair AP feeding the same tile to both DoubleRow slots
        return bass.AP(ap.tensor, ap.offset, [ap.ap[0], [0, 2], ap.ap[1]])

    def pair_ap(t, off, stride, width, inner=1):
        a = t[:]
        return bass.AP(a.tensor, a.offset + off,
                       [a.ap[0], [stride, 2], [inner, width]])

    with tile.TileContext(nc) as tc, ExitStack() as top:
        const = top.enter_context(tc.tile_pool(name="const", bufs=1))
        qtp = top.enter_context(tc.tile_pool(name="qtp", bufs=2))
        ptp = top.enter_context(tc.tile_pool(name="ptp", bufs=36))
        atp = top.enter_context(tc.tile_pool(name="atp", bufs=3))
        rcp = top.enter_context(tc.tile_pool(name="rcp", bufs=3))
        ojp = top.enter_context(tc.tile_pool(name="ojp", bufs=6))
        ps = top.enter_context(tc.tile_pool(name="ps", bufs=1, space="PSUM"))

        # ---------- input DMAs (order = DMA-engine priority) ----------
        xTh_sb, xTl_sb = [], []
        for ch in range(NCH):
            xTh_sb.append(const.tile([P, CT * NC2], E4, tag=f"xTh{ch}", name=f"xTh{ch}"))
            xTl_sb.append(const.tile([P, CT * NC2], E4, tag=f"xTl{ch}", name=f"xTl{ch}"))
        idx_sb = const.tile([P, NKV // 16], I16, tag="idx")
        nc.sync.dma_start(idx_sb[:], idx[:])
        nc.sync.dma_start(xTh_sb[0][:], xTh[0, :, :])
        nc.sync.dma_start(xTl_sb[0][:], xTl[0, :, :])
        wqh_sb, wql_sb = [], []
        for g in range(G):
            wqh_sb.append(const.tile([P, CT * P], E4, tag=f"wqh{g}",
                                     name=f"wqh{g}"))
            wql_sb.append(const.tile([P, CT * P], E4, tag=f"wql{g}",
                                     name=f"wql{g}"))
        nc.sync.dma_start(wqh_sb[0][:], wqh[0, :, :])
        nc.sync.dma_start(wql_sb[0][:], wql[0, :, :])
        nc.sync.dma_start(wqh_sb[1][:], wqh[1, :, :])
        nc.sync.dma_start(wql_sb[1][:], wql[1, :, :])
        wkh_sb = const.tile([P, CT * C], E4, tag="wkh")
        nc.sync.dma_start(wkh_sb[:], wkh[:])
        wkl_sb = const.tile([P, CT * C], E4, tag="wkl")
        nc.sync.dma_start(wkl_sb[:], wkl[:])

        warm = const.tile([P, NC2], BF16, tag="warm")
        nc.gpsimd.memset(warm[:], 0.0)

        # transposing gather (bf16), then hi/lo fp8 planes on DVE
        xkvT = const.tile([P, CT * NKV], BF16, tag="xkvT")
        xkvT3 = xkvT[:].rearrange("p (i m) -> p i m", i=CT)
        nc.gpsimd.dma_gather(
            out_ap=xkvT3, in_ap=xr[:], idxs_ap=idx_sb[:], num_idxs=NKV,
            num_idxs_reg=NKV, elem_size=C, transpose=True)
        xkh = const.tile([P, CT * NKV], E4, tag="xkh")
        nc.vector.tensor_copy(xkh[:], xkvT[:])
        xkl = const.tile([P, CT * NKV], E4, tag="xkl")
        cut = 4 * NKV
        nc.vector.tensor_sub(xkl[:, 0:cut], xkvT[:, 0:cut], xkh[:, 0:cut])
        nc.gpsimd.tensor_sub(xkl[:, cut:], xkvT[:, cut:], xkh[:, cut:])

        vaug_sb = []
        for k in range(MCH):
            va = const.tile([P, H * (HD + 1)], BF16, tag=f"vaug{k}", name=f"vaug{k}")
            nc.gpsimd.memset(va[:], 1.0 / AS)
            vaug_sb.append(va)

        for g in range(2, G):
            nc.sync.dma_start(wqh_sb[g][:], wqh[g, :, :])
            nc.sync.dma_start(wql_sb[g][:], wql[g, :, :])
        wvh_sb = const.tile([P, CT * C], E4, tag="wvh")
        nc.sync.dma_start(wvh_sb[:], wvh[:])
        wvl_sb = const.tile([P, CT * C], E4, tag="wvl")
        nc.sync.dma_start(wvl_sb[:], wvl[:])
        pb_sb = const.tile([P, CT], F32, tag="pb")
        nc.sync.dma_start(pb_sb[:], pb[:])
        for ch in range(1, NCH):
            nc.sync.dma_start(xTh_sb[ch][:], xTh[ch, :, :])
            nc.sync.dma_start(xTl_sb[ch][:], xTl[ch, :, :])
        wph_sb = const.tile([P, CT * C], E4, tag="wph")
        nc.sync.dma_start(wph_sb[:], wph[:])
        wpl_sb = const.tile([P, CT * C], E4, tag="wpl")
        nc.sync.dma_start(wpl_sb[:], wpl[:])

        # kT hi/lo planes in one tile: hi at cols [0, G*NKV), lo above
        kTP = const.tile([P, 2 * G * NKV], E4, tag="kTP")
        kTP2 = kTP[:].rearrange("p (two gk) -> p two gk", two=2)

        # attn transpose windows (2-chunk pipeline, manually double-buffered)
        attnT16 = [const.tile([P, CT * NC2], BF16, tag=f"aT16_{i}", name=f"aT16_{i}")
                   for i in range(2)]
        attnT8h = [const.tile([P, CT * NC2], E4, tag=f"aT8h_{i}", name=f"aT8h_{i}")
                   for i in range(2)]
        attnT8l = [const.tile([P, CT * NC2], E4, tag=f"aT8l_{i}", name=f"aT8l_{i}")
                   for i in range(2)]

        # ---------- 3-term DoubleRow chain ----------
        def chain9(o, mh, ml, sh, sl, mstride, mwidth, moff, sstride, soff):
            n = 0
            for mv, st in ((mh, sh), (mh, sl), (ml, sh)):
                for j in range(CT // 2):
                    m_ap = pair_ap(mv, moff + 2 * j * mstride, mstride, mwidth)
                    s_ap = pair_ap(st, soff + 2 * j * sstride, sstride, P)
                    nc.tensor.matmul(o, s_ap, m_ap, start=(n == 0), stop=(n == 8),
                                     perf_mode=DR)
                    n += 1

        # ---------- q / k / v projections ----------
        def emit_qp(ch, g):
            qp = ps.tile([P, NC2], F32, tag="mm", bufs=2, name=f"qp{ch}_{g}")
            chain9(qp[:], xTh_sb[ch], xTl_sb[ch], wqh_sb[g], wql_sb[g],
                   NC2, NC2, 0, P, 0)
            qt = qtp.tile([P, NC2], E4, tag=f"qT{g}", name=f"qt{ch}_{g}")
            nc.vector.tensor_copy(qt[:], qp[:])   # stores 32*q as fp8
            return qt

        def emit_kt(g):
            kp = ps.tile([P, NKV], F32, tag="mm", bufs=2, name=f"kp{g}")
            chain9(kp[:], xkh, xkl, wkh_sb, wkl_sb, NKV, NKV, 0, C, g * P)
            hi = kTP[:, g * NKV:(g + 1) * NKV]
            nc.vector.tensor_copy(hi, kp[:])
            nc.vector.tensor_tensor(
                kTP[:, G * NKV + g * NKV:G * NKV + (g + 1) * NKV],
                kp[:], hi, Sub)

        def emit_vaug(k):
            va3 = vaug_sb[k][:].rearrange("p (h x) -> p h x", x=HD + 1)
            for half in range(2):
                vp = ps.tile([P, CT * HD], F32, tag="mm", bufs=2,
                             name=f"vp{k}_{half}")
                # moving = wv column pair, stationary = xkv key-tile pair
                chain9(vp[:], wvh_sb, wvl_sb, xkh, xkl,
                       C, CT * HD, half * CT * HD, NKV, k * P)
                nc.vector.tensor_scalar(
                    va3[:, CT * half:CT * half + CT, 0:HD],
                    vp[:].rearrange("p (h x) -> p h x", x=HD),
                    1.0 / WS, None, Mult)

        # p-state warm-up: cheap matmuls keep PE busy until inputs land
        for w in range(10):
            d = ps.tile([P, NC2], F32, tag="mm", bufs=2, name=f"warm{w}")
            nc.tensor.matmul(d[:], warm[0:4, 0:P], warm[0:4, :],
                             start=True, stop=True)

        qT = [None] * G
        qT[0] = emit_qp(0, 0)
        qT[1] = emit_qp(0, 1)
        for w in range(10, 16):
            d = ps.tile([P, NC2], F32, tag="mm", bufs=2, name=f"warm{w}")
            nc.tensor.matmul(d[:], warm[0:4, 0:P], warm[0:4, :],
                             start=True, stop=True)
        emit_kt(0)
        emit_kt(1)

        # ---------- per-chunk emitters ----------
        def emit_scores_exp(ch, g, par, pts_h):
            base = par * HD
            for half in range(2):
                st2 = ps.tile([P, 2 * NC2], F32, tag="st", bufs=2,
                              name=f"st{ch}_{g}_{par}_{half}")
                for hh in range(2):
                    kc = 2 * half + hh
                    s_ap = kTP2[base:base + HD, :,
                                g * NKV + kc * P:g * NKV + (kc + 1) * P]
                    m_ap = dup2(qT[g][base:base + HD, :])
                    nc.tensor.matmul(st2[:, hh * NC2:(hh + 1) * NC2], s_ap, m_ap,
                                     start=True, stop=True, perf_mode=DR)
                pt = ptp.tile([P, 2 * NC2], BF16, tag="pt",
                              name=f"pt{ch}_{2 * g + par}_{half}")
                nc.scalar.activation(pt[:], st2[:], Exp, scale=SCALE / (WS * WS))
                pts_h.append(pt)

        def emit_ov_ttile(ch, pts, ttile):
            # 12 head chains into two 1-bank group tiles, then batch-normalize
            grpA = ps.tile([P, 455], F32, tag="gA", bufs=1,
                           name=f"grpA{ch}_{ttile}")
            grpB = ps.tile([P, 325], F32, tag="gB", bufs=1,
                           name=f"grpB{ch}_{ttile}")
            gA, gB = grpA[:], grpB[:]
            for h in range(H):
                g_ap = gA if h < 7 else gB
                off = h * 65 if h < 7 else (h - 7) * 65
                o = bass.AP(g_ap.tensor, g_ap.offset + off,
                            [g_ap.ap[0], [1, 65]])
                for kc in range(MCH):
                    half, hh = kc // 2, kc % 2
                    p_sl = pts[h][half][:, hh * NC2 + ttile * P:
                                        hh * NC2 + (ttile + 1) * P]
                    nc.tensor.matmul(o, p_sl,
                                     vaug_sb[kc][:, h * 65:(h + 1) * 65],
                                     start=(kc == 0), stop=(kc == MCH - 1))
            rc = rcp.tile([P, 12], F32, tag="rc", name=f"rc{ch}_{ttile}")
            d1 = bass.AP(gA.tensor, gA.offset + 64, [gA.ap[0], [65, 7]])
            d2 = bass.AP(gB.tensor, gB.offset + 64, [gB.ap[0], [65, 5]])
            nc.vector.reciprocal(rc[:, 0:7], d1)
            nc.vector.reciprocal(rc[:, 7:12], d2)
            at = atp.tile([P, C], BF16, tag="at16", name=f"at{ch}_{ttile}")
            rap = rc[:]
            v1 = bass.AP(gA.tensor, gA.offset, [gA.ap[0], [65, 7], [1, HD]])
            v2 = bass.AP(gB.tensor, gB.offset, [gB.ap[0], [65, 5], [1, HD]])
            r1 = bass.AP(rap.tensor, rap.offset, [rap.ap[0], [1, 7], [0, HD]])
            r2 = bass.AP(rap.tensor, rap.offset + 7, [rap.ap[0], [1, 5], [0, HD]])
            nc.vector.tensor_tensor(
                at[:, 0:7 * HD].rearrange("p (h x) -> p h x", x=HD), v1, r1, Mult)
            nc.vector.tensor_tensor(
                at[:, 7 * HD:C].rearrange("p (h x) -> p h x", x=HD), v2, r2, Mult)
            # at16 [tok, feat] -> attnT16 [feat, tok] via XBAR transpose DMA
            o3 = attnT16[ch % 2][:].rearrange("p (j t) -> p j t", j=CT)
            nc.sync.dma_start(o3[:, :, ttile * P:(ttile + 1) * P], at[:],
                              transpose=True)

        def emit_casts(ch, ttile, on_act=False, lo_dve=False):
            w = ch % 2
            sl = (slice(None), slice(None),
                  slice(ttile * P, (ttile + 1) * P))
            a16 = attnT16[w][:].rearrange("p (j t) -> p j t", j=CT)[sl]
            a8h = attnT8h[w][:].rearrange("p (j t) -> p j t", j=CT)[sl]
            a8l = attnT8l[w][:].rearrange("p (j t) -> p j t", j=CT)[sl]
            if on_act:
                nc.scalar.activation(a8h, a16,
                                     mybir.ActivationFunctionType.Copy)
            else:
                nc.vector.tensor_copy(a8h, a16)
            if lo_dve:
                nc.vector.tensor_sub(a8l, a16, a8h)
            else:
                nc.gpsimd.tensor_sub(a8l, a16, a8h)

        def emit_proj_one(ch, j):
            w = ch % 2
            pp = ps.tile([P, NC2], F32, tag="mm", bufs=2, name=f"pp{ch}_{j}")
            chain9(pp[:], attnT8h[w], attnT8l[w], wph_sb, wpl_sb,
                   NC2, NC2, 0, C, j * P)
            oj = ojp.tile([P, NC2], BF16, tag="oj", name=f"oj{ch}_{j}")
            nc.vector.tensor_scalar(oj[:], pp[:], 1.0 / (WS * AS),
                                    pb_sb[:, j:j + 1], Mult, Add)
            nc.sync.dma_start(out[:, j, ch, :], oj[:])

        # ---------- main loop: chunk ch scores/exp + chunk ch-1 deferred ----
        def emit_q0(g):
            qT[g] = emit_qp(0, g)

        deferred = [
            (lambda: emit_kt(2), 1), (lambda: emit_kt(3), 1),
            (lambda: emit_q0(2), 1), (lambda: emit_kt(4), 1),
            (lambda: emit_q0(3), 1), (lambda: emit_kt(5), 1),
            (lambda: emit_q0(4), 1), (lambda: emit_q0(5), 1),
        ]
        deferred += [(lambda k=k: emit_vaug(k), 1) for k in range(MCH)]
        for ch in range(NCH):
            qT_next = []
            pts = [[] for _ in range(H)]
            for g in range(G):
                for par in range(2):
                    emit_scores_exp(ch, g, par, pts[2 * g + par])
                    if par == 0 and ch + 1 < NCH:
                        qT_next.append(emit_qp(ch + 1, g))
                    slots_left = 12 - (2 * g + par)
                    npop = (len(deferred) + slots_left - 1) // slots_left
                    for _ in range(min(npop, len(deferred))):
                        fn, _pe = deferred.pop(0)
                        fn()
            if ch + 1 < NCH:
                units = []
                for t in range(4):
                    units.append((lambda c=ch, p=pts, t=t:
                                  emit_ov_ttile(c, p, t), 1))
                    units.append((lambda c=ch, t=t: emit_casts(c, t), 0))
                units += [(lambda c=ch, j=j: emit_proj_one(c, j), 1)
                          for j in range(CT)]
                deferred = units
                qT = qT_next

        # ---------- tail drain: last chunk, per-ttile pipelined proj ----------
        ch = NCH - 1
        w = ch % 2
        for fn, _pe in deferred:
            fn()
        for t in range(4):
            emit_ov_ttile(ch, pts, t)
            emit_casts(ch, t, on_act=True, lo_dve=(t % 2 == 1))
        pps = [ps.tile([P, 1024], F32, tag="st", bufs=2, name=f"ppd{i}")
               for i in range(2)]
        pps += [ps.tile([P, NC2], F32, tag="mm", bufs=2, name=f"ppm{i}")
                for i in range(2)]

        def ppslice(j, lo, hi):
            if j < 4:
                return pps[j // 2][:, (j % 2) * NC2 + lo:(j % 2) * NC2 + hi]
            return pps[j - 2][:, lo:hi]

        def quarter(j, t):
            o = ppslice(j, t * P, (t + 1) * P)
            n = 0
            for mv, st_ in ((attnT8h[w], wph_sb), (attnT8h[w], wpl_sb),
                            (attnT8l[w], wph_sb)):
                for m in range(CT // 2):
                    m_ap = pair_ap(mv, 2 * m * NC2 + t * P, NC2, P)
                    s_ap = pair_ap(st_, j * P + 2 * m * C, C, P)
                    nc.tensor.matmul(o, s_ap, m_ap, start=(n == 0),
                                     stop=(n == 8), perf_mode=DR)
                    n += 1

        for t in range(3):
            for j in range(CT):
                quarter(j, t)
        for j in range(CT):
            quarter(j, 3)
            oj = ojp.tile([P, NC2], BF16, tag="oj", name=f"ojd{j}")
            src_ap = ppslice(j, 0, NC2)
            if j % 2 == 0:
                nc.scalar.activation(
                    oj[:], src_ap, mybir.ActivationFunctionType.Identity,
                    scale=1.0 / (WS * AS), bias=pb_sb[:, j:j + 1])
            else:
                nc.vector.tensor_scalar(oj[:], src_ap, 1.0 / (WS * AS),
                                        pb_sb[:, j:j + 1], Mult, Add)
            nc.sync.dma_start(out[:, j, ch, :], oj[:])

    nc.compile()
    return nc


def _get_nc():
    if "nc" not in _CACHE:
        _CACHE["nc"] = _build()
    return _CACHE["nc"]


def _prep_core_inputs(x, key_ind, q_w, kv_w, proj_w, proj_b):
    """Build the 8 per-core input maps (fp8 hi/lo planes + bf16 gather src)."""
    bf16 = ml_dtypes.bfloat16
    e4 = ml_dtypes.float8_e4m3

    def split8(a):
        a = np.asarray(a, np.float32)
        hi = np.clip(a, -240, 240).astype(e4)
        lo = (a - hi.astype(np.float32)).astype(e4)
        return hi, lo

    def wT_pack(w, perm=None):
        # [C(out), C(in)] weight -> transposed blocks [P, CT*C] f32
        wT = w.T.astype(np.float32)
        if perm is not None:
            wT = wT[perm]
        return np.ascontiguousarray(
            wT.reshape(CT, P, C).transpose(1, 0, 2).reshape(P, CT * C))


    # wq repacked per head pair: [G, P, CT*128], pre-scaled x32
    wqp = np.ascontiguousarray(
        wT_pack(q_w).reshape(P, CT, G, P).transpose(2, 0, 1, 3)
        .reshape(G, P, CT * P)) * WS
    wqh, wql = split8(wqp)
    kvwT3 = kv_w.T.astype(np.float32).reshape(C, H, 2 * HD)
    wkh, wkl = split8(wT_pack(np.ascontiguousarray(
        kvwT3[:, :, :HD].reshape(C, C)).T) * WS)
    wvh, wvl = split8(wT_pack(np.ascontiguousarray(
        kvwT3[:, :, HD:].reshape(C, C)).T) * WS)
    wph, wpl = split8(wT_pack(proj_w) * WS)
    pbp = np.ascontiguousarray(proj_b.astype(np.float32).reshape(CT, P).T)
    x = np.asarray(x, np.float32).astype(bf16).astype(np.float32)

    def xT_pack(plane):
        return np.ascontiguousarray(
            plane.T.reshape(CT, P, NCH, NC2).transpose(2, 1, 0, 3)
            .reshape(NCH, P, CT * NC2))

    in_maps = []
    for b in range(B):
        xb = x[b]                                   # [N, C] (bf16 values)
        xh, xl = split8(xb)
        xTh_b = xT_pack(xh.astype(np.float32)).astype(e4)
        xTl_b = xT_pack(xl.astype(np.float32)).astype(e4)
        idxb = np.ascontiguousarray(np.tile(
            np.asarray(key_ind[b]).astype(np.int16).reshape(NKV // 16, 16).T,
            (8, 1)))
        in_maps.append({
            "xTh": xTh_b, "xTl": xTl_b, "xr": xb.astype(bf16), "idx": idxb,
            "wqh": wqh, "wql": wql, "wkh": wkh, "wkl": wkl,
            "wvh": wvh, "wvl": wvl, "wph": wph, "wpl": wpl, "pb": pbp,
        })
    return in_maps


def kernel(x, key_ind, q_w, kv_w, proj_w, proj_b, _trace=False, _results=None):
    from concourse.bass_utils import run_bass_kernel_spmd

    nc = _get_nc()
    in_maps = _prep_core_inputs(x, key_ind, q_w, kv_w, proj_w, proj_b)
    res = run_bass_kernel_spmd(nc, in_maps, core_ids=list(range(B)), trace=_trace)
    if _results is not None:
        _results.append(res)
    outp = np.empty((B, N, C), dtype=np.float32)
    for b in range(B):
        o = res.results[b]["out"].astype(np.float32)   # [P, CT, NCH, NC2]
        outp[b] = o.transpose(2, 3, 1, 0).reshape(N, C)
    return outp


# revision 24
# speedup vs baseline: 1.0002x; 1.0002x over previous
"""Sparse attention kernel for Trainium2 (8 NeuronCores, data-parallel over batch).

Reference computation (per batch row b):
    q    = x @ q_w.T                                  [N, C]
    xkv  = x[key_ind]                                 [NKV, C]
    kv   = xkv @ kv_w.T -> per-head k, v              [NKV, 2C]
    attn = softmax((q*scale) @ k.T) @ v               [N, C]
    out  = attn @ proj_w.T + proj_b                   [N, C]

Compute plan (per core = one batch row):
  - All dense projections (q/k/v/proj) run as 3-term hi/lo fp8 DoubleRow
    chains: (a_hi + a_lo) @ (w_hi + w_lo) with the lo*lo term dropped.
    Each DoubleRow matmul contracts TWO 128-deep k-chunks at 0.5 cyc/row,
    so a 768-deep contraction runs in 9 DR matmuls = 0.75x the bf16 cost
    at near-bf16 accuracy. Weights are pre-scaled x32 host-side so their
    hi/lo planes stay clear of e4m3's subnormal range; descales fold into
    the exp scale and the evac ops.
  - Scores: one DoubleRow matmul per key-tile with stationary (k_hi,k_lo)
    and the moving q8 duplicated via a stride-0 AP (2x PE throughput; the
    single-fp8 q quantization is the only real precision loss, ~1.4e-2).
  - OV runs transposed: stationary = p-tile [keys, 128tok], moving =
    vaug [keys, 65] (v columns + a 1/16 ones column) -> out [tok, 65]
    costs 65 cycles/matmul instead of 512. Denominator lands in col 64;
    normalization is a batched strided reciprocal + stride-0-broadcast
    multiply on DVE producing at16 = 16*attn (x16 keeps the hi/lo split
    of attn clear of subnormals).
  - at16 [tok, feat] returns to [feat, tok] via XBAR transpose DMA
    (14ns per 16x128 tile, on the DMA queues, zero engine cost), then
    Pool splits it into fp8 hi/lo planes for the proj chains.
  - Engine balance: Act does ONLY the exp stream (its ~82us floor
    dominates), all PSUM evacs are DVE, all SBUF-side casts are Pool
    (Pool cannot touch PSUM). Deferred OV/cast/proj work from chunk ch
    drains through the 12 (head) slots of chunk ch+1's score loop.
"""
import os
import sys

sys.path.insert(0, "/opt/trn_rl_repo")

import numpy as np  # noqa: E402
import ml_dtypes  # noqa: E402

B, N, C = 8, 2048, 768
NKV = 512
H = 12
HD = C // H          # 64
SCALE = HD ** -0.5
P = 128
CT = C // P          # 6 feature tiles
NC2 = 512            # token chunk
NCH = N // NC2       # 4 chunks
MCH = NKV // P       # 4 key chunks
G = H // 2           # 6 head pairs
WS = 32.0            # weight pre-scale (host side)
AS = 16.0            # attn pre-scale (via 1/16 ones column)

_CACHE = {}


def _build():
    import concourse.bass as bass
    import concourse.mybir as mybir
    import concourse.tile as tile
    from concourse import bacc
    from contextlib import ExitStack

    F32 = mybir.dt.float32
    BF16 = mybir.dt.bfloat16
    E4 = mybir.dt.float8e4
    I16 = mybir.dt.int16
    Exp = mybir.ActivationFunctionType.Exp
    DR = mybir.MatmulPerfMode.DoubleRow
    Mult = mybir.AluOpType.mult
    Add = mybir.AluOpType.add
    Sub = mybir.AluOpType.subtract

    nc = bacc.Bacc("TRN2", target_bir_lowering=False, debug=False, num_devices=8)

    xTh = nc.dram_tensor("xTh", [NCH, P, CT * NC2], E4, kind="ExternalInput")
    xTl = nc.dram_tensor("xTl", [NCH, P, CT * NC2], E4, kind="ExternalInput")
    xr = nc.dram_tensor("xr", [N, C], BF16, kind="ExternalInput")
    idx = nc.dram_tensor("idx", [P, NKV // 16], I16, kind="ExternalInput")
    wqh = nc.dram_tensor("wqh", [G, P, CT * P], E4, kind="ExternalInput")
    wql = nc.dram_tensor("wql", [G, P, CT * P], E4, kind="ExternalInput")
    wkh = nc.dram_tensor("wkh", [P, CT * C], E4, kind="ExternalInput")
    wkl = nc.dram_tensor("wkl", [P, CT * C], E4, kind="ExternalInput")
    wvh = nc.dram_tensor("wvh", [P, CT * C], E4, kind="ExternalInput")
    wvl = nc.dram_tensor("wvl", [P, CT * C], E4, kind="ExternalInput")
    wph = nc.dram_tensor("wph", [P, CT * C], E4, kind="ExternalInput")
    wpl = nc.dram_tensor("wpl", [P, CT * C], E4, kind="ExternalInput")
    pb = nc.dram_tensor("pb", [P, CT], F32, kind="ExternalInput")
    out = nc.dram_tensor("out", [P, CT, NCH, NC2], BF16, kind="ExternalOutput")

    def dup2(ap):
        # moving-pair AP feeding the same tile to both DoubleRow slots
        return bass.AP(ap.tensor, ap.offset, [ap.ap[0], [0, 2], ap.ap[1]])

    def pair_ap(t, off, stride, width, inner=1):
        a = t[:]
        return bass.AP(a.tensor, a.offset + off,
                       [a.ap[0], [stride, 2], [inner, width]])

    with tile.TileContext(nc) as tc, ExitStack() as top:
        const = top.enter_context(tc.tile_pool(name="const", bufs=1))
        qtp = top.enter_context(tc.tile_pool(name="qtp", bufs=2))
        ptp = top.enter_context(tc.tile_pool(name="ptp", bufs=36))
        atp = top.enter_context(tc.tile_pool(name="atp", bufs=3))
        rcp = top.enter_context(tc.tile_pool(name="rcp", bufs=3))
        ojp = top.enter_context(tc.tile_pool(name="ojp", bufs=6))
        ps = top.enter_context(tc.tile_pool(name="ps", bufs=1, space="PSUM"))

        # ---------- input DMAs (order = DMA-engine priority) ----------
        xTh_sb, xTl_sb = [], []
        for ch in range(NCH):
            xTh_sb.append(const.tile([P, CT * NC2], E4, tag=f"xTh{ch}", name=f"xTh{ch}"))
            xTl_sb.append(const.tile([P, CT * NC2], E4, tag=f"xTl{ch}", name=f"xTl{ch}"))
        idx_sb = const.tile([P, NKV // 16], I16, tag="idx")
        nc.sync.dma_start(idx_sb[:], idx[:])
        nc.sync.dma_start(xTh_sb[0][:], xTh[0, :, :])
        nc.sync.dma_start(xTl_sb[0][:], xTl[0, :, :])
        wqh_sb, wql_sb = [], []
        for g in range(G):
            wqh_sb.append(const.tile([P, CT * P], E4, tag=f"wqh{g}",
                                     name=f"wqh{g}"))
            wql_sb.append(const.tile([P, CT * P], E4, tag=f"wql{g}",
                                     name=f"wql{g}"))
        nc.sync.dma_start(wqh_sb[0][:], wqh[0, :, :])
        nc.sync.dma_start(wql_sb[0][:], wql[0, :, :])
        nc.sync.dma_start(wqh_sb[1][:], wqh[1, :, :])
        nc.sync.dma_start(wql_sb[1][:], wql[1, :, :])
        wkh_sb = const.tile([P, CT * C], E4, tag="wkh")
        nc.sync.dma_start(wkh_sb[:], wkh[:])
        wkl_sb = const.tile([P, CT * C], E4, tag="wkl")
        nc.sync.dma_start(wkl_sb[:], wkl[:])

        warm = const.tile([P, NC2], BF16, tag="warm")
        nc.gpsimd.memset(warm[:], 0.0)

        # transposing gather (bf16), then hi/lo fp8 planes on DVE
        xkvT = const.tile([P, CT * NKV], BF16, tag="xkvT")
        xkvT3 = xkvT[:].rearrange("p (i m) -> p i m", i=CT)
        nc.gpsimd.dma_gather(
            out_ap=xkvT3, in_ap=xr[:], idxs_ap=idx_sb[:], num_idxs=NKV,
            num_idxs_reg=NKV, elem_size=C, transpose=True)
        xkh = const.tile([P, CT * NKV], E4, tag="xkh")
        nc.vector.tensor_copy(xkh[:], xkvT[:])
        xkl = const.tile([P, CT * NKV], E4, tag="xkl")
        cut = 4 * NKV
        nc.vector.tensor_sub(xkl[:, 0:cut], xkvT[:, 0:cut], xkh[:, 0:cut])
        nc.gpsimd.tensor_sub(xkl[:, cut:], xkvT[:, cut:], xkh[:, cut:])

        vaug_sb = []
        for k in range(MCH):
            va = const.tile([P, H * (HD + 1)], BF16, tag=f"vaug{k}", name=f"vaug{k}")
            nc.gpsimd.memset(va[:], 1.0 / AS)
            vaug_sb.append(va)

        for g in range(2, G):
            nc.sync.dma_start(wqh_sb[g][:], wqh[g, :, :])
            nc.sync.dma_start(wql_sb[g][:], wql[g, :, :])
        wvh_sb = const.tile([P, CT * C], E4, tag="wvh")
        nc.sync.dma_start(wvh_sb[:], wvh[:])
        wvl_sb = const.tile([P, CT * C], E4, tag="wvl")
        nc.sync.dma_start(wvl_sb[:], wvl[:])
        pb_sb = const.tile([P, CT], F32, tag="pb")
        nc.sync.dma_start(pb_sb[:], pb[:])
        for ch in range(1, NCH):
            nc.sync.dma_start(xTh_sb[ch][:], xTh[ch, :, :])
            nc.sync.dma_start(xTl_sb[ch][:], xTl[ch, :, :])
        wph_sb = const.tile([P, CT * C], E4, tag="wph")
        nc.sync.dma_start(wph_sb[:], wph[:])
        wpl_sb = const.tile([P, CT * C], E4, tag="wpl")
        nc.sync.dma_start(wpl_sb[:], wpl[:])

        # kT hi/lo planes in one tile: hi at cols [0, G*NKV), lo above
        kTP = const.tile([P, 2 * G * NKV], E4, tag="kTP")
        kTP2 = kTP[:].rearrange("p (two gk) -> p two gk", two=2)

        # attn transpose windows (2-chunk pipeline, manually double-buffered)
        attnT16 = [const.tile([P, CT * NC2], BF16, tag=f"aT16_{i}", name=f"aT16_{i}")
                   for i in range(2)]
        attnT8h = [const.tile([P, CT * NC2], E4, tag=f"aT8h_{i}", name=f"aT8h_{i}")
                   for i in range(2)]
        attnT8l = [const.tile([P, CT * NC2], E4, tag=f"aT8l_{i}", name=f"aT8l_{i}")
                   for i in range(2)]

        # ---------- 3-term DoubleRow chain ----------
        def chain9(o, mh, ml, sh, sl, mstride, mwidth, moff, sstride, soff):
            n = 0
            for mv, st in ((mh, sh), (mh, sl), (ml, sh)):
                for j in range(CT // 2):
                    m_ap = pair_ap(mv, moff + 2 * j * mstride, mstride, mwidth)
                    s_ap = pair_ap(st, soff + 2 * j * sstride, sstride, P)
                    nc.tensor.matmul(o, s_ap, m_ap, start=(n == 0), stop=(n == 8),
                                     perf_mode=DR)
                    n += 1

        # ---------- q / k / v projections ----------
        def emit_qp(ch, g):
            qp = ps.tile([P, NC2], F32, tag="mm", bufs=2, name=f"qp{ch}_{g}")
            chain9(qp[:], xTh_sb[ch], xTl_sb[ch], wqh_sb[g], wql_sb[g],
                   NC2, NC2, 0, P, 0)
            qt = qtp.tile([P, NC2], E4, tag=f"qT{g}", name=f"qt{ch}_{g}")
            nc.vector.tensor_copy(qt[:], qp[:])   # stores 32*q as fp8
            return qt

        def emit_kt(g):
            kp = ps.tile([P, NKV], F32, tag="mm", bufs=2, name=f"kp{g}")
            chain9(kp[:], xkh, xkl, wkh_sb, wkl_sb, NKV, NKV, 0, C, g * P)
            hi = kTP[:, g * NKV:(g + 1) * NKV]
            nc.vector.tensor_copy(hi, kp[:])
            nc.vector.tensor_tensor(
                kTP[:, G * NKV + g * NKV:G * NKV + (g + 1) * NKV],
                kp[:], hi, Sub)

        def emit_vaug(k):
            va3 = vaug_sb[k][:].rearrange("p (h x) -> p h x", x=HD + 1)
            for half in range(2):
                vp = ps.tile([P, CT * HD], F32, tag="mm", bufs=2,
                             name=f"vp{k}_{half}")
                # moving = wv column pair, stationary = xkv key-tile pair
                chain9(vp[:], wvh_sb, wvl_sb, xkh, xkl,
                       C, CT * HD, half * CT * HD, NKV, k * P)
                nc.vector.tensor_scalar(
                    va3[:, CT * half:CT * half + CT, 0:HD],
                    vp[:].rearrange("p (h x) -> p h x", x=HD),
                    1.0 / WS, None, Mult)

        # p-state warm-up: cheap matmuls keep PE busy until inputs land
        for w in range(10):
            d = ps.tile([P, NC2], F32, tag="mm", bufs=2, name=f"warm{w}")
            nc.tensor.matmul(d[:], warm[0:4, 0:P], warm[0:4, :],
                             start=True, stop=True)

        qT = [None] * G
        qT[0] = emit_qp(0, 0)
        qT[1] = emit_qp(0, 1)
        emit_kt(0)
        emit_kt(1)

        # ---------- per-chunk emitters ----------
        def emit_scores_exp(ch, g, par, pts_h):
            base = par * HD
            for half in range(2):
                st2 = ps.tile([P, 2 * NC2], F32, tag="st", bufs=2,
                              name=f"st{ch}_{g}_{par}_{half}")
                for hh in range(2):
                    kc = 2 * half + hh
                    s_ap = kTP2[base:base + HD, :,
                                g * NKV + kc * P:g * NKV + (kc + 1) * P]
                    m_ap = dup2(qT[g][base:base + HD, :])
                    nc.tensor.matmul(st2[:, hh * NC2:(hh + 1) * NC2], s_ap, m_ap,
                                     start=True, stop=True, perf_mode=DR)
                pt = ptp.tile([P, 2 * NC2], BF16, tag="pt",
                              name=f"pt{ch}_{2 * g + par}_{half}")
                nc.scalar.activation(pt[:], st2[:], Exp, scale=SCALE / (WS * WS))
                pts_h.append(pt)

        def emit_ov_ttile(ch, pts, ttile):
            # 12 head chains into two 1-bank group tiles, then batch-normalize
            grpA = ps.tile([P, 455], F32, tag="gA", bufs=1,
                           name=f"grpA{ch}_{ttile}")
            grpB = ps.tile([P, 325], F32, tag="gB", bufs=1,
                           name=f"grpB{ch}_{ttile}")
            gA, gB = grpA[:], grpB[:]
            for h in range(H):
                g_ap = gA if h < 7 else gB
                off = h * 65 if h < 7 else (h - 7) * 65
                o = bass.AP(g_ap.tensor, g_ap.offset + off,
                            [g_ap.ap[0], [1, 65]])
                for kc in range(MCH):
                    half, hh = kc // 2, kc % 2
                    p_sl = pts[h][half][:, hh * NC2 + ttile * P:
                                        hh * NC2 + (ttile + 1) * P]
                    nc.tensor.matmul(o, p_sl,
                                     vaug_sb[kc][:, h * 65:(h + 1) * 65],
                                     start=(kc == 0), stop=(kc == MCH - 1))
            rc = rcp.tile([P, 12], F32, tag="rc", name=f"rc{ch}_{ttile}")
            d1 = bass.AP(gA.tensor, gA.offset + 64, [gA.ap[0], [65, 7]])
            d2 = bass.AP(gB.tensor, gB.offset + 64, [gB.ap[0], [65, 5]])
            nc.vector.reciprocal(rc[:, 0:7], d1)
            nc.vector.reciprocal(rc[:, 7:12], d2)
            at = atp.tile([P, C], BF16, tag="at16", name=f"at{ch}_{ttile}")
            rap = rc[:]
            v1 = bass.AP(gA.tensor, gA.offset, [gA.ap[0], [65, 7], [1, HD]])
            v2 = bass.AP(gB.tensor, gB.offset, [gB.ap[0], [65, 5], [1, HD]])
            r1 = bass.AP(rap.tensor, rap.offset, [rap.ap[0], [1, 7], [0, HD]])
            r2 = bass.AP(rap.tensor, rap.offset + 7, [rap.ap[0], [1, 5], [0, HD]])
            nc.vector.tensor_tensor(
                at[:, 0:7 * HD].rearrange("p (h x) -> p h x", x=HD), v1, r1, Mult)
            nc.vector.tensor_tensor(
                at[:, 7 * HD:C].rearrange("p (h x) -> p h x", x=HD), v2, r2, Mult)
            # at16 [tok, feat] -> attnT16 [feat, tok] via XBAR transpose DMA
            o3 = attnT16[ch % 2][:].rearrange("p (j t) -> p j t", j=CT)
            nc.sync.dma_start(o3[:, :, ttile * P:(ttile + 1) * P], at[:],
                              transpose=True)

        def emit_casts(ch, ttile, on_act=False, lo_dve=False):
            w = ch % 2
            sl = (slice(None), slice(None),
                  slice(ttile * P, (ttile + 1) * P))
            a16 = attnT16[w][:].rearrange("p (j t) -> p j t", j=CT)[sl]
            a8h = attnT8h[w][:].rearrange("p (j t) -> p j t", j=CT)[sl]
            a8l = attnT8l[w][:].rearrange("p (j t) -> p j t", j=CT)[sl]
            if on_act:
                nc.scalar.activation(a8h, a16,
                                     mybir.ActivationFunctionType.Copy)
            else:
                nc.vector.tensor_copy(a8h, a16)
            if lo_dve:
                nc.vector.tensor_sub(a8l, a16, a8h)
            else:
                nc.gpsimd.tensor_sub(a8l, a16, a8h)

        def emit_proj_one(ch, j):
            w = ch % 2
            pp = ps.tile([P, NC2], F32, tag="mm", bufs=2, name=f"pp{ch}_{j}")
            chain9(pp[:], attnT8h[w], attnT8l[w], wph_sb, wpl_sb,
                   NC2, NC2, 0, C, j * P)
            oj = ojp.tile([P, NC2], BF16, tag="oj", name=f"oj{ch}_{j}")
            nc.vector.tensor_scalar(oj[:], pp[:], 1.0 / (WS * AS),
                                    pb_sb[:, j:j + 1], Mult, Add)
            nc.sync.dma_start(out[:, j, ch, :], oj[:])

        # ---------- main loop: chunk ch scores/exp + chunk ch-1 deferred ----
        def emit_q0(g):
            qT[g] = emit_qp(0, g)

        deferred = [
            (lambda: emit_kt(2), 1), (lambda: emit_kt(3), 1),
            (lambda: emit_q0(2), 1), (lambda: emit_kt(4), 1),
            (lambda: emit_q0(3), 1), (lambda: emit_kt(5), 1),
            (lambda: emit_q0(4), 1), (lambda: emit_q0(5), 1),
        ]
        deferred += [(lambda k=k: emit_vaug(k), 1) for k in range(MCH)]
        for ch in range(NCH):
            qT_next = []
            pts = [[] for _ in range(H)]
            for g in range(G):
                for par in range(2):
                    emit_scores_exp(ch, g, par, pts[2 * g + par])
                    if par == 0 and ch + 1 < NCH:
                        qT_next.append(emit_qp(ch + 1, g))
                    slots_left = 12 - (2 * g + par)
                    npop = (len(deferred) + slots_left - 1) // slots_left
                    for _ in range(min(npop, len(deferred))):
                        fn, _pe = deferred.pop(0)
                        fn()
            if ch + 1 < NCH:
                units = []
                for t in range(4):
                    units.append((lambda c=ch, p=pts, t=t:
                                  emit_ov_ttile(c, p, t), 1))
                    units.append((lambda c=ch, t=t: emit_casts(c, t), 0))
                units += [(lambda c=ch, j=j: emit_proj_one(c, j), 1)
                          for j in range(CT)]
                deferred = units
                qT = qT_next

        # ---------- tail drain: last chunk, per-ttile pipelined proj ----------
        ch = NCH - 1
        w = ch % 2
        for fn, _pe in deferred:
            fn()
        for t in range(4):
            emit_ov_ttile(ch, pts, t)
            emit_casts(ch, t, on_act=True, lo_dve=(t % 2 == 1))
        pps = [ps.tile([P, 1024], F32, tag="st", bufs=2, name=f"ppd{i}")
               for i in range(2)]
        pps += [ps.tile([P, NC2], F32, tag="mm", bufs=2, name=f"ppm{i}")
                for i in range(2)]

        def ppslice(j, lo, hi):
            if j < 4:
                return pps[j // 2][:, (j % 2) * NC2 + lo:(j % 2) * NC2 + hi]
            return pps[j - 2][:, lo:hi]

        def quarter(j, t):
            o = ppslice(j, t * P, (t + 1) * P)
            n = 0
            for mv, st_ in ((attnT8h[w], wph_sb), (attnT8h[w], wpl_sb),
                            (attnT8l[w], wph_sb)):
                for m in range(CT // 2):
                    m_ap = pair_ap(mv, 2 * m * NC2 + t * P, NC2, P)
                    s_ap = pair_ap(st_, j * P + 2 * m * C, C, P)
                    nc.tensor.matmul(o, s_ap, m_ap, start=(n == 0),
                                     stop=(n == 8), perf_mode=DR)
                    n += 1

        for t in range(3):
            for j in range(CT):
                quarter(j, t)
        for j in range(CT):
            quarter(j, 3)
            oj = ojp.tile([P, NC2], BF16, tag="oj", name=f"ojd{j}")
            src_ap = ppslice(j, 0, NC2)
            if j % 2 == 0:
                nc.scalar.activation(
                    oj[:], src_ap, mybir.ActivationFunctionType.Identity,
                    scale=1.0 / (WS * AS), bias=pb_sb[:, j:j + 1])
            else:
                nc.vector.tensor_scalar(oj[:], src_ap, 1.0 / (WS * AS),
                                        pb_sb[:, j:j + 1], Mult, Add)
            nc.sync.dma_start(out[:, j, ch, :], oj[:])

    nc.compile()
    return nc


def _get_nc():
    if "nc" not in _CACHE:
        _CACHE["nc"] = _build()
    return _CACHE["nc"]


def _prep_core_inputs(x, key_ind, q_w, kv_w, proj_w, proj_b):
    """Build the 8 per-core input maps (fp8 hi/lo planes + bf16 gather src)."""
    bf16 = ml_dtypes.bfloat16
    e4 = ml_dtypes.float8_e4m3

    def split8(a):
        a = np.asarray(a, np.float32)
        hi = np.clip(a, -240, 240).astype(e4)
        lo = (a - hi.astype(np.float32)).astype(e4)
        return hi, lo

    def wT_pack(w, perm=None):
        # [C(out), C(in)] weight -> transposed blocks [P, CT*C] f32
        wT = w.T.astype(np.float32)
        if perm is not None:
            wT = wT[perm]
        return np.ascontiguousarray(
            wT.reshape(CT, P, C).transpose(1, 0, 2).reshape(P, CT * C))


    # wq repacked per head pair: [G, P, CT*128], pre-scaled x32
    wqp = np.ascontiguousarray(
        wT_pack(q_w).reshape(P, CT, G, P).transpose(2, 0, 1, 3)
        .reshape(G, P, CT * P)) * WS
    wqh, wql = split8(wqp)
    kvwT3 = kv_w.T.astype(np.float32).reshape(C, H, 2 * HD)
    wkh, wkl = split8(wT_pack(np.ascontiguousarray(
        kvwT3[:, :, :HD].reshape(C, C)).T) * WS)
    wvh, wvl = split8(wT_pack(np.ascontiguousarray(
        kvwT3[:, :, HD:].reshape(C, C)).T) * WS)
    wph, wpl = split8(wT_pack(proj_w) * WS)
    pbp = np.ascontiguousarray(proj_b.astype(np.float32).reshape(CT, P).T)
    x = np.asarray(x, np.float32).astype(bf16).astype(np.float32)

    def xT_pack(plane):
        return np.ascontiguousarray(
            plane.T.reshape(CT, P, NCH, NC2).transpose(2, 1, 0, 3)
            .reshape(NCH, P, CT * NC2))

    in_maps = []
    for b in range(B):
        xb = x[b]                                   # [N, C] (bf16 values)
        xh, xl = split8(xb)
        xTh_b = xT_pack(xh.astype(np.float32)).astype(e4)
        xTl_b = xT_pack(xl.astype(np.float32)).astype(e4)
        idxb = np.ascontiguousarray(np.tile(
            np.asarray(key_ind[b]).astype(np.int16).reshape(NKV // 16, 16).T,
            (8, 1)))
        in_maps.append({
            "xTh": xTh_b, "xTl": xTl_b, "xr": xb.astype(bf16), "idx": idxb,
            "wqh": wqh, "wql": wql, "wkh": wkh, "wkl": wkl,
            "wvh": wvh, "wvl": wvl, "wph": wph, "wpl": wpl, "pb": pbp,
        })
    return in_maps


def kernel(x, key_ind, q_w, kv_w, proj_w, proj_b, _trace=False, _results=None):
    from concourse.bass_utils import run_bass_kernel_spmd

    nc = _get_nc()
    in_maps = _prep_core_inputs(x, key_ind, q_w, kv_w, proj_w, proj_b)
    res = run_bass_kernel_spmd(nc, in_maps, core_ids=list(range(B)), trace=_trace)
    if _results is not None:
        _results.append(res)
    outp = np.empty((B, N, C), dtype=np.float32)
    for b in range(B):
        o = res.results[b]["out"].astype(np.float32)   # [P, CT, NCH, NC2]
        outp[b] = o.transpose(2, 3, 1, 0).reshape(N, C)
    return outp
